# revision 19
# baseline (speedup 1.0000x reference)
"""NT-Xent loss kernel for 8 Trainium2 NeuronCores.

Math (matches the reference):
  Z = concat(z_i, z_j).reshape(8192, 128); r = row-l2-normalize(Z)
  sim = r @ r.T                                  (8192 x 8192)
  row i: S_i   = sum_j exp(2*sim[i, j])          (full row, incl. self)
         d_i   = exp(2*sim[i, i])                (self term)
         p_i   = exp(2*sim[i, pair(i)]),  pair(i) = (i + 4096) % 8192
  loss_i = log(S_i - d_i + p_i) - log(p_i)
  loss   = mean_i(loss_i)

Sharding: rows are split across 8 cores (1024 rows each). Every core gets
the full Z, but ROTATED so its own rows come first; this makes the
self-diagonal land at local columns [0, 1024) and the pair diagonal at
[4096, 5120) on every core, so one SPMD program works for all cores.
Each core emits its 1024 per-row losses; the host sums them (the scalar
all-reduce) and divides by 2N.

Host-side staging: z is supplied pre-rotated AND pre-tiled as
z_sh[p, t*128 + d] = z_rot[t*128 + p, d] so every DMA is fully
contiguous per partition.

Per-core pipeline:
  1. DMA z (fp32) in 8 sub-chunks of 1024 rows.
  2. Row norms on DVE only: square, reduce, rsqrt via Quake seed + 2
     fused Newton steps (no ACT Sqrt -> no activation-table thrash).
  3. Scale rows to unit norm, cast bf16, one batched DMA-xbar transpose
     per sub-chunk into RT[d, 8192].
  4. PE warm-up matmuls (HAM clock gate) timed right before the main loop.
  5. Main loop: per (2048-col chunk x 128-row block): 4 bf16 matmuls
     (512 cols each) into PSUM, one ACT Exp (scale=2) in-place with
     fused row-sum (accum_out). Self/pair diag extracted from the exp'd
     PSUM via multiply-with-identity + reduce on DVE.
  6. Epilogue: S - d + p, Ln, subtract, DMA out [128, 8] losses.
"""

import sys

import numpy as np

sys.path.insert(0, "/opt/trn_rl_repo")

from contextlib import ExitStack  # noqa: E402

import concourse.bass as bass  # noqa: E402
import concourse.tile as tile  # noqa: E402
from concourse import bacc, mybir  # noqa: E402
from concourse.bass_utils import run_bass_kernel_spmd  # noqa: E402

P = 128
N_CORES = 8
NROWS = 8192  # 2N
D = 128
ROWS_PER_CORE = NROWS // N_CORES  # 1024
RB = ROWS_PER_CORE // P  # 8 row blocks per core
G = 4  # column chunk groups (main loop)
CH = NROWS // G  # 2048 columns per chunk
SC = 8  # normalization sub-chunks
SCR = NROWS // SC  # 1024 rows per sub-chunk
TPS = SCR // P  # 8 row-tiles per sub-chunk
MM_N = 512  # matmul moving free dim (one PSUM bank)
WARMUP_MM = 10  # PE warm-up matmuls (~3.4us busy to flip the HAM gate)

F32 = mybir.dt.float32
BF16 = mybir.dt.bfloat16
U32 = mybir.dt.uint32
AF = mybir.ActivationFunctionType
OP = mybir.AluOpType
AX = mybir.AxisListType

_CACHE = {}


def _broadcast_last(ap: bass.AP, n: int) -> bass.AP:
    """Append a stride-0 dim of size n to an AP (free-axis broadcast)."""
    return bass.AP(tensor=ap.tensor, offset=ap.offset, ap=[*ap.ap, [0, n]])


def _build_nc():
    nc = bacc.Bacc(
        "TRN2", target_bir_lowering=False, debug=False, num_devices=N_CORES
    )
    z = nc.dram_tensor("z", [P, NROWS], F32, kind="ExternalInput").ap()
    ident = nc.dram_tensor("ident", [P, P], F32, kind="ExternalInput").ap()
    out = nc.dram_tensor("loss8", [P, RB], F32, kind="ExternalOutput").ap()

    with tile.TileContext(nc) as tc, ExitStack() as ctx:
        zpool = ctx.enter_context(tc.tile_pool(name="zpool", bufs=3))
        sqpool = ctx.enter_context(tc.tile_pool(name="sqpool", bufs=2))
        znpool = ctx.enter_context(tc.tile_pool(name="znpool", bufs=3))
        small = ctx.enter_context(tc.tile_pool(name="small", bufs=3))
        singles = ctx.enter_context(tc.tile_pool(name="singles", bufs=1))
        psum = ctx.enter_context(tc.tile_pool(name="psum", bufs=2, space="PSUM"))

        # Persistent transposed normalized representation: RT[d, n]
        rt = singles.tile([P, NROWS], BF16)

        Ssum = singles.tile([P, RB * G], F32)  # per (row, chunk) partial sums
        d8 = singles.tile([P, RB], F32)  # exp(2*self)
        p8 = singles.tile([P, RB], F32)  # exp(2*pair)

        # ---- normalization: 8 pipelined sub-chunks of 1024 rows ----
        for c in range(SC):
            zt = zpool.tile([P, TPS, D], F32)
            if c == 0:
                # split the first (critical-path) load into two parallel DMAs
                half = SCR // 2
                nc.gpsimd.dma_start(out=zt[:, : TPS // 2, :], in_=z[:, 0:half])
                nc.gpsimd.dma_start(
                    out=zt[:, TPS // 2 :, :], in_=z[:, half:SCR]
                )
            else:
                nc.gpsimd.dma_start(out=zt[:], in_=z[:, c * SCR : (c + 1) * SCR])
            if c == 1:
                # identity for diag extraction; off the critical first load
                sb_ident = singles.tile([P, P], F32)
                nc.gpsimd.dma_start(out=sb_ident[:], in_=ident)
            sq = sqpool.tile([P, TPS, D], F32)
            nc.vector.tensor_mul(sq[:], zt[:], zt[:])
            ss = small.tile([P, TPS], F32)
            nc.vector.tensor_reduce(ss[:], sq[:], axis=AX.X, op=OP.add)

            # u = 1/sqrt(ss): Quake seed + 2 fused Newton iterations.
            # DVE's scalar ALU promotes to f32, so build the seed as
            # (0xBE6EB3BE - bits) via float mult/add, then integer >>1.
            u = small.tile([P, TPS], F32)
            tmp = small.tile([P, TPS], F32)
            nc.vector.tensor_scalar(
                tmp[:].bitcast(U32),
                ss[:].bitcast(U32),
                -1.0,
                float(0xBE6EB3BE),
                OP.mult,
                OP.add,
            )
            nc.vector.tensor_scalar(
                u[:].bitcast(U32),
                tmp[:].bitcast(U32),
                1,
                None,
                OP.logical_shift_right,
            )
            for _ in range(2):
                # t = (y*y * -0.5) * ss ; y = (t + 1.5) * y
                nc.vector.tensor_mul(tmp[:], u[:], u[:])
                nc.vector.scalar_tensor_tensor(
                    out=tmp[:], in0=tmp[:], scalar=-0.5, in1=ss[:],
                    op0=OP.mult, op1=OP.mult,
                )
                nc.vector.scalar_tensor_tensor(
                    out=u[:], in0=tmp[:], scalar=1.5, in1=u[:],
                    op0=OP.add, op1=OP.mult,
                )

            zn = znpool.tile([P, TPS, D], BF16)
            nc.vector.tensor_mul(zn[:], zt[:], _broadcast_last(u[:], D))
            # batched xbar transpose for the whole 1024-col sub-chunk:
            # out[a, b, c] = in[c, b*128 + a]  ->  rt[d, t*128+p] = zn[p, t, d]
            nc.sync.dma_start(
                out=rt[:, c * SCR : (c + 1) * SCR].rearrange(
                    "d (t p) -> d t p", p=P
                ),
                in_=zn[:],
                transpose=True,
            )

        # ---- main loop: sim chunk -> exp -> row-sum ----
        expool = ctx.enter_context(tc.tile_pool(name="expool", bufs=4))
        for g in range(G):
            for rb in range(RB):
                ps = psum.tile([P, CH], F32)
                for s in range(CH // MM_N):
                    nc.tensor.matmul(
                        ps[:, s * MM_N : (s + 1) * MM_N],
                        rt[:, rb * P : (rb + 1) * P],
                        rt[:, g * CH + s * MM_N : g * CH + (s + 1) * MM_N],
                        start=True,
                        stop=True,
                    )
                if g in (0, 2):
                    # chunks carrying the self/pair diagonal: exp to SBUF
                    # scratch (PSUM frees immediately; extraction decouples)
                    ex = expool.tile([P, CH], F32)
                    nc.scalar.activation(
                        out=ex[:],
                        in_=ps[:],
                        func=AF.Exp,
                        scale=2.0,
                        accum_out=Ssum[:, rb * G + g : rb * G + g + 1],
                    )
                    # diag extract off the critical path: multiply the
                    # 128x128 diag block by identity, reduce along free
                    dst = (d8 if g == 0 else p8)[:, rb : rb + 1]
                    dummy = small.tile([P, P], F32)
                    nc.vector.tensor_mul(
                        dummy[:], ex[:, rb * P : rb * P + P], sb_ident[:]
                    )
                    nc.vector.tensor_reduce(dst, dummy[:], axis=AX.X, op=OP.add)
                else:
                    nc.scalar.activation(
                        out=ps[:],
                        in_=ps[:],
                        func=AF.Exp,
                        scale=2.0,
                        accum_out=Ssum[:, rb * G + g : rb * G + g + 1],
                    )

        # ---- epilogue ----
        S8 = singles.tile([P, RB], F32)
        nc.vector.tensor_reduce(
            S8[:], Ssum[:].rearrange("p (r g) -> p r g", g=G), axis=AX.X, op=OP.add
        )
        # S8 <- S8 - d8 + p8
        nc.vector.scalar_tensor_tensor(
            out=S8[:], in0=d8[:], scalar=-1.0, in1=S8[:], op0=OP.mult, op1=OP.add
        )
        nc.vector.tensor_add(S8[:], S8[:], p8[:])
        lse = singles.tile([P, RB], F32)
        nc.scalar.activation(out=lse[:], in_=S8[:], func=AF.Ln)
        p2 = singles.tile([P, RB], F32)
        nc.scalar.activation(out=p2[:], in_=p8[:], func=AF.Ln)
        loss8 = singles.tile([P, RB], F32)
        nc.vector.scalar_tensor_tensor(
            out=loss8[:], in0=p2[:], scalar=-1.0, in1=lse[:], op0=OP.mult, op1=OP.add
        )
        nc.sync.dma_start(out=out, in_=loss8[:])

    nc.compile()
    return nc


def get_nc():
    if "nc" not in _CACHE:
        _CACHE["nc"] = _build_nc()
    return _CACHE["nc"]


def make_in_maps(z_i: np.ndarray, z_j: np.ndarray):
    Z = np.concatenate(
        [
            np.asarray(z_i, np.float32).reshape(NROWS // 2, D),
            np.asarray(z_j, np.float32).reshape(NROWS // 2, D),
        ],
        axis=0,
    )
    ident = np.eye(P, dtype=np.float32)
    in_maps = []
    for k in range(N_CORES):
        zk = np.roll(Z, -k * ROWS_PER_CORE, axis=0)
        # z_sh[p, t*128+d] = zk[t*128+p, d]: contiguous per-partition DMA
        zsh = np.ascontiguousarray(
            zk.reshape(NROWS // P, P, D).transpose(1, 0, 2)
        ).reshape(P, NROWS)
        in_maps.append({"z": zsh, "ident": ident})
    return in_maps


def run_full(z_i: np.ndarray, z_j: np.ndarray, trace: bool = False):
    nc = get_nc()
    in_maps = make_in_maps(z_i, z_j)
    res = run_bass_kernel_spmd(nc, in_maps, list(range(N_CORES)), trace=trace)
    total = 0.0
    for k in range(N_CORES):
        total += float(np.asarray(res.results[k]["loss8"], np.float64).sum())
    loss = np.float32(total / NROWS)
    return loss, res


def kernel(z_i: np.ndarray, z_j: np.ndarray) -> np.ndarray:
    loss, _ = run_full(z_i, z_j, trace=False)
    return np.asarray(loss, dtype=np.float32)


# revision 21
# speedup vs baseline: 1.1476x; 1.1476x over previous
"""NT-Xent loss kernel for 8 Trainium2 NeuronCores.

Math (matches the reference):
  Z = concat(z_i, z_j).reshape(8192, 128); r = row-l2-normalize(Z)
  sim = r @ r.T                                  (8192 x 8192)
  row i: S_i   = sum_j exp(2*sim[i, j])          (full row, incl. self)
         d_i   = exp(2*sim[i, i])                (self term)
         p_i   = exp(2*sim[i, pair(i)]),  pair(i) = (i + 4096) % 8192
  loss_i = log(S_i - d_i + p_i) - log(p_i)
  loss   = mean_i(loss_i)

Sharding: rows are split across 8 cores (1024 rows each). Every core gets
the full Z, but ROTATED so its own rows come first; this makes the
self-diagonal land at local columns [0, 1024) and the pair diagonal at
[4096, 5120) on every core, so one SPMD program works for all cores.
Each core emits its 1024 per-row losses; the host sums them (the scalar
all-reduce) and divides by 2N.

Host-side staging: z is supplied pre-rotated AND pre-tiled as
z_sh[p, t*128 + d] = z_rot[t*128 + p, d] so every DMA is fully
contiguous per partition.

Per-core pipeline:
  1. DMA z (fp32) in 8 sub-chunks of 1024 rows.
  2. Row norms on DVE only: square, reduce, rsqrt via Quake seed + 2
     fused Newton steps (no ACT Sqrt -> no activation-table thrash).
  3. Scale rows to unit norm, cast bf16, one batched DMA-xbar transpose
     per sub-chunk into RT[d, 8192].
  4. PE warm-up matmuls (HAM clock gate) timed right before the main loop.
  5. Main loop: per (2048-col chunk x 128-row block): 4 bf16 matmuls
     (512 cols each) into PSUM, one ACT Exp (scale=2) in-place with
     fused row-sum (accum_out). Self/pair diag extracted from the exp'd
     PSUM via multiply-with-identity + reduce on DVE.
  6. Epilogue: S - d + p, Ln, subtract, DMA out [128, 8] losses.
"""

import sys

import numpy as np

sys.path.insert(0, "/opt/trn_rl_repo")

from contextlib import ExitStack  # noqa: E402

import concourse.bass as bass  # noqa: E402
import concourse.tile as tile  # noqa: E402
from concourse import bacc, mybir  # noqa: E402
from concourse.bass_utils import run_bass_kernel_spmd  # noqa: E402

P = 128
N_CORES = 8
NROWS = 8192  # 2N
D = 128
ROWS_PER_CORE = NROWS // N_CORES  # 1024
RB = ROWS_PER_CORE // P  # 8 row blocks per core
G = 4  # column chunk groups (main loop)
CH = NROWS // G  # 2048 columns per chunk
SC = 8  # normalization sub-chunks
SCR = NROWS // SC  # 1024 rows per sub-chunk
TPS = SCR // P  # 8 row-tiles per sub-chunk
MM_N = 512  # matmul moving free dim (one PSUM bank)
WARMUP_MM = 10  # PE warm-up matmuls (~3.4us busy to flip the HAM gate)

F32 = mybir.dt.float32
BF16 = mybir.dt.bfloat16
U32 = mybir.dt.uint32
AF = mybir.ActivationFunctionType
OP = mybir.AluOpType
AX = mybir.AxisListType

_CACHE = {}


def _broadcast_last(ap: bass.AP, n: int) -> bass.AP:
    """Append a stride-0 dim of size n to an AP (free-axis broadcast)."""
    return bass.AP(tensor=ap.tensor, offset=ap.offset, ap=[*ap.ap, [0, n]])


def _build_nc():
    nc = bacc.Bacc(
        "TRN2", target_bir_lowering=False, debug=False, num_devices=N_CORES
    )
    z = nc.dram_tensor("z", [P, NROWS], F32, kind="ExternalInput").ap()
    ident = nc.dram_tensor("ident", [P, P], F32, kind="ExternalInput").ap()
    out = nc.dram_tensor("loss8", [P, RB], F32, kind="ExternalOutput").ap()

    with tile.TileContext(nc) as tc, ExitStack() as ctx:
        zpool = ctx.enter_context(tc.tile_pool(name="zpool", bufs=SC))
        sqpool = ctx.enter_context(tc.tile_pool(name="sqpool", bufs=2))
        znpool = ctx.enter_context(tc.tile_pool(name="znpool", bufs=3))
        small = ctx.enter_context(tc.tile_pool(name="small", bufs=3))
        singles = ctx.enter_context(tc.tile_pool(name="singles", bufs=1))
        psum = ctx.enter_context(tc.tile_pool(name="psum", bufs=2, space="PSUM"))

        # Persistent transposed normalized representation: RT[d, n]
        rt = singles.tile([P, NROWS], BF16)

        Ssum = singles.tile([P, RB * G], F32)  # per (row, chunk) partial sums
        d8 = singles.tile([P, RB], F32)  # exp(2*self)
        p8 = singles.tile([P, RB], F32)  # exp(2*pair)

        # ---- normalization: 8 pipelined sub-chunks of 1024 rows ----
        # All loads are emitted first so no queue-blocking wait (e.g. an
        # xbar transpose waiting on zn) can delay a later load's dispatch.
        zts = []
        for c in range(SC):
            zt = zpool.tile([P, TPS, D], F32)
            if c == 0:
                # split the first (critical-path) load into two parallel DMAs
                half = SCR // 2
                nc.sync.dma_start(out=zt[:, : TPS // 2, :], in_=z[:, 0:half])
                nc.sync.dma_start(out=zt[:, TPS // 2 :, :], in_=z[:, half:SCR])
            else:
                nc.sync.dma_start(out=zt[:], in_=z[:, c * SCR : (c + 1) * SCR])
            zts.append(zt)
        sb_ident = singles.tile([P, P], F32)
        nc.sync.dma_start(out=sb_ident[:], in_=ident)

        for c in range(SC):
            zt = zts[c]
            sq = sqpool.tile([P, TPS, D], F32)
            nc.vector.tensor_mul(sq[:], zt[:], zt[:])
            ss = small.tile([P, TPS], F32)
            nc.vector.tensor_reduce(ss[:], sq[:], axis=AX.X, op=OP.add)

            # u = 1/sqrt(ss): Quake seed + 2 fused Newton iterations.
            # DVE's scalar ALU promotes to f32, so build the seed as
            # (0xBE6EB3BE - bits) via float mult/add, then integer >>1.
            u = small.tile([P, TPS], F32)
            tmp = small.tile([P, TPS], F32)
            nc.vector.tensor_scalar(
                tmp[:].bitcast(U32),
                ss[:].bitcast(U32),
                -1.0,
                float(0xBE6EB3BE),
                OP.mult,
                OP.add,
            )
            nc.vector.tensor_scalar(
                u[:].bitcast(U32),
                tmp[:].bitcast(U32),
                1,
                None,
                OP.logical_shift_right,
            )
            for _ in range(2):
                # t = (y*y * -0.5) * ss ; y = (t + 1.5) * y
                nc.vector.tensor_mul(tmp[:], u[:], u[:])
                nc.vector.scalar_tensor_tensor(
                    out=tmp[:], in0=tmp[:], scalar=-0.5, in1=ss[:],
                    op0=OP.mult, op1=OP.mult,
                )
                nc.vector.scalar_tensor_tensor(
                    out=u[:], in0=tmp[:], scalar=1.5, in1=u[:],
                    op0=OP.add, op1=OP.mult,
                )

            zn = znpool.tile([P, TPS, D], BF16)
            nc.vector.tensor_mul(zn[:], zt[:], _broadcast_last(u[:], D))
            # batched xbar transpose for the whole 1024-col sub-chunk:
            # out[a, b, c] = in[c, b*128 + a]  ->  rt[d, t*128+p] = zn[p, t, d]
            nc.sync.dma_start(
                out=rt[:, c * SCR : (c + 1) * SCR].rearrange(
                    "d (t p) -> d t p", p=P
                ),
                in_=zn[:],
                transpose=True,
            )

        # ---- main loop: sim chunk -> exp -> row-sum ----
        expool = ctx.enter_context(tc.tile_pool(name="expool", bufs=4))
        for g in range(G):
            for rb in range(RB):
                ps = psum.tile([P, CH], F32)
                for s in range(CH // MM_N):
                    nc.tensor.matmul(
                        ps[:, s * MM_N : (s + 1) * MM_N],
                        rt[:, rb * P : (rb + 1) * P],
                        rt[:, g * CH + s * MM_N : g * CH + (s + 1) * MM_N],
                        start=True,
                        stop=True,
                    )
                if g in (0, 2):
                    # chunks carrying the self/pair diagonal: exp to SBUF
                    # scratch (PSUM frees immediately; extraction decouples)
                    ex = expool.tile([P, CH], F32)
                    nc.scalar.activation(
                        out=ex[:],
                        in_=ps[:],
                        func=AF.Exp,
                        scale=2.0,
                        accum_out=Ssum[:, rb * G + g : rb * G + g + 1],
                    )
                    # diag extract off the critical path: multiply the
                    # 128x128 diag block by identity, reduce along free
                    dst = (d8 if g == 0 else p8)[:, rb : rb + 1]
                    dummy = small.tile([P, P], F32)
                    nc.vector.tensor_mul(
                        dummy[:], ex[:, rb * P : rb * P + P], sb_ident[:]
                    )
                    nc.vector.tensor_reduce(dst, dummy[:], axis=AX.X, op=OP.add)
                else:
                    nc.scalar.activation(
                        out=ps[:],
                        in_=ps[:],
                        func=AF.Exp,
                        scale=2.0,
                        accum_out=Ssum[:, rb * G + g : rb * G + g + 1],
                    )

        # ---- epilogue ----
        S8 = singles.tile([P, RB], F32)
        nc.vector.tensor_reduce(
            S8[:], Ssum[:].rearrange("p (r g) -> p r g", g=G), axis=AX.X, op=OP.add
        )
        # S8 <- S8 - d8 + p8
        nc.vector.scalar_tensor_tensor(
            out=S8[:], in0=d8[:], scalar=-1.0, in1=S8[:], op0=OP.mult, op1=OP.add
        )
        nc.vector.tensor_add(S8[:], S8[:], p8[:])
        lse = singles.tile([P, RB], F32)
        nc.scalar.activation(out=lse[:], in_=S8[:], func=AF.Ln)
        p2 = singles.tile([P, RB], F32)
        nc.scalar.activation(out=p2[:], in_=p8[:], func=AF.Ln)
        loss8 = singles.tile([P, RB], F32)
        nc.vector.scalar_tensor_tensor(
            out=loss8[:], in0=p2[:], scalar=-1.0, in1=lse[:], op0=OP.mult, op1=OP.add
        )
        nc.sync.dma_start(out=out, in_=loss8[:])

    nc.compile()
    return nc


def get_nc():
    if "nc" not in _CACHE:
        _CACHE["nc"] = _build_nc()
    return _CACHE["nc"]


def make_in_maps(z_i: np.ndarray, z_j: np.ndarray):
    Z = np.concatenate(
        [
            np.asarray(z_i, np.float32).reshape(NROWS // 2, D),
            np.asarray(z_j, np.float32).reshape(NROWS // 2, D),
        ],
        axis=0,
    )
    ident = np.eye(P, dtype=np.float32)
    in_maps = []
    for k in range(N_CORES):
        zk = np.roll(Z, -k * ROWS_PER_CORE, axis=0)
        # z_sh[p, t*128+d] = zk[t*128+p, d]: contiguous per-partition DMA
        zsh = np.ascontiguousarray(
            zk.reshape(NROWS // P, P, D).transpose(1, 0, 2)
        ).reshape(P, NROWS)
        in_maps.append({"z": zsh, "ident": ident})
    return in_maps


def run_full(z_i: np.ndarray, z_j: np.ndarray, trace: bool = False):
    nc = get_nc()
    in_maps = make_in_maps(z_i, z_j)
    res = run_bass_kernel_spmd(nc, in_maps, list(range(N_CORES)), trace=trace)
    total = 0.0
    for k in range(N_CORES):
        total += float(np.asarray(res.results[k]["loss8"], np.float64).sum())
    loss = np.float32(total / NROWS)
    return loss, res


def kernel(z_i: np.ndarray, z_j: np.ndarray) -> np.ndarray:
    loss, _ = run_full(z_i, z_j, trace=False)
    return np.asarray(loss, dtype=np.float32)


# revision 23
# speedup vs baseline: 1.1528x; 1.0045x over previous
"""NT-Xent loss kernel for 8 Trainium2 NeuronCores.

Math (matches the reference):
  Z = concat(z_i, z_j).reshape(8192, 128); r = row-l2-normalize(Z)
  sim = r @ r.T                                  (8192 x 8192)
  row i: S_i   = sum_j exp(2*sim[i, j])          (full row, incl. self)
         d_i   = exp(2*sim[i, i])                (self term)
         p_i   = exp(2*sim[i, pair(i)]),  pair(i) = (i + 4096) % 8192
  loss_i = log(S_i - d_i + p_i) - log(p_i)
  loss   = mean_i(loss_i)

Sharding: rows are split across 8 cores (1024 rows each). Every core gets
the full Z, but ROTATED so its own rows come first; this makes the
self-diagonal land at local columns [0, 1024) and the pair diagonal at
[4096, 5120) on every core, so one SPMD program works for all cores.
Each core emits its 1024 per-row losses; the host sums them (the scalar
all-reduce) and divides by 2N.

Host-side staging: z is supplied pre-rotated AND pre-tiled as
z_sh[p, t*128 + d] = z_rot[t*128 + p, d] so every DMA is fully
contiguous per partition.

Per-core pipeline:
  1. DMA z (fp32) in 8 sub-chunks of 1024 rows.
  2. Row norms on DVE only: square, reduce, rsqrt via Quake seed + 2
     fused Newton steps (no ACT Sqrt -> no activation-table thrash).
  3. Scale rows to unit norm, cast bf16, one batched DMA-xbar transpose
     per sub-chunk into RT[d, 8192].
  4. Main loop: per (2048-col chunk x 128-row block): 4 bf16 matmuls
     (512 cols each) into PSUM, one ACT Exp (scale=2) with fused row-sum
     (accum_out); diag-carrying chunks write exp to SBUF scratch so the
     PSUM slot frees immediately and the diag extraction (multiply by
     identity + reduce on DVE) runs off the critical path.
  5. Epilogue: S - d + p, Ln, subtract, DMA out [128, 8] losses.
"""

import sys

import numpy as np

sys.path.insert(0, "/opt/trn_rl_repo")

from contextlib import ExitStack  # noqa: E402

import concourse.bass as bass  # noqa: E402
import concourse.tile as tile  # noqa: E402
from concourse import bacc, mybir  # noqa: E402
from concourse.bass_utils import run_bass_kernel_spmd  # noqa: E402

P = 128
N_CORES = 8
NROWS = 8192  # 2N
D = 128
ROWS_PER_CORE = NROWS // N_CORES  # 1024
RB = ROWS_PER_CORE // P  # 8 row blocks per core
G = 4  # column chunk groups (main loop)
CH = NROWS // G  # 2048 columns per chunk
SC = 8  # normalization sub-chunks
SCR = NROWS // SC  # 1024 rows per sub-chunk
TPS = SCR // P  # 8 row-tiles per sub-chunk
MM_N = 512  # matmul moving free dim (one PSUM bank)

F32 = mybir.dt.float32
BF16 = mybir.dt.bfloat16
U32 = mybir.dt.uint32
AF = mybir.ActivationFunctionType
OP = mybir.AluOpType
AX = mybir.AxisListType

_CACHE = {}


def _broadcast_last(ap: bass.AP, n: int) -> bass.AP:
    """Append a stride-0 dim of size n to an AP (free-axis broadcast)."""
    return bass.AP(tensor=ap.tensor, offset=ap.offset, ap=[*ap.ap, [0, n]])


def _build_nc():
    nc = bacc.Bacc(
        "TRN2", target_bir_lowering=False, debug=False, num_devices=N_CORES
    )
    z = nc.dram_tensor("z", [P, NROWS], F32, kind="ExternalInput").ap()
    ident = nc.dram_tensor("ident", [P, P], F32, kind="ExternalInput").ap()
    out = nc.dram_tensor("loss8", [P, RB], F32, kind="ExternalOutput").ap()

    with tile.TileContext(nc) as tc, ExitStack() as ctx:
        zpool = ctx.enter_context(tc.tile_pool(name="zpool", bufs=SC))
        sqpool = ctx.enter_context(tc.tile_pool(name="sqpool", bufs=2))
        znpool = ctx.enter_context(tc.tile_pool(name="znpool", bufs=3))
        small = ctx.enter_context(tc.tile_pool(name="small", bufs=3))
        singles = ctx.enter_context(tc.tile_pool(name="singles", bufs=1))
        psum = ctx.enter_context(tc.tile_pool(name="psum", bufs=2, space="PSUM"))

        # Persistent transposed normalized representation: RT[d, n]
        rt = singles.tile([P, NROWS], BF16)

        Ssum = singles.tile([P, RB * G], F32)  # per (row, chunk) partial sums
        d8 = singles.tile([P, RB], F32)  # exp(2*self)
        p8 = singles.tile([P, RB], F32)  # exp(2*pair)

        # ---- normalization: 8 pipelined sub-chunks of 1024 rows ----
        # All loads are emitted first so no queue-blocking wait (e.g. an
        # xbar transpose waiting on zn) can delay a later load's dispatch.
        zts = []
        for c in range(SC):
            zt = zpool.tile([P, TPS, D], F32)
            if c == 0:
                # split the first (critical-path) load into two parallel DMAs
                half = SCR // 2
                nc.sync.dma_start(out=zt[:, : TPS // 2, :], in_=z[:, 0:half])
                nc.sync.dma_start(out=zt[:, TPS // 2 :, :], in_=z[:, half:SCR])
            else:
                nc.sync.dma_start(out=zt[:], in_=z[:, c * SCR : (c + 1) * SCR])
            zts.append(zt)
        sb_ident = singles.tile([P, P], F32)
        nc.sync.dma_start(out=sb_ident[:], in_=ident)

        for c in range(SC):
            zt = zts[c]
            sq = sqpool.tile([P, TPS, D], F32)
            nc.vector.tensor_mul(sq[:], zt[:], zt[:])
            ss = small.tile([P, TPS], F32)
            nc.vector.tensor_reduce(ss[:], sq[:], axis=AX.X, op=OP.add)

            # u = 1/sqrt(ss): Quake seed + 2 fused Newton iterations.
            # DVE's scalar ALU promotes to f32, so build the seed as
            # (0xBE6EB3BE - bits) via float mult/add, then integer >>1.
            u = small.tile([P, TPS], F32)
            tmp = small.tile([P, TPS], F32)
            nc.vector.tensor_scalar(
                tmp[:].bitcast(U32),
                ss[:].bitcast(U32),
                -1.0,
                float(0xBE6EB3BE),
                OP.mult,
                OP.add,
            )
            nc.vector.tensor_scalar(
                u[:].bitcast(U32),
                tmp[:].bitcast(U32),
                1,
                None,
                OP.logical_shift_right,
            )
            for _ in range(2):
                # t = (y*y * -0.5) * ss ; y = (t + 1.5) * y
                nc.vector.tensor_mul(tmp[:], u[:], u[:])
                nc.vector.scalar_tensor_tensor(
                    out=tmp[:], in0=tmp[:], scalar=-0.5, in1=ss[:],
                    op0=OP.mult, op1=OP.mult,
                )
                nc.vector.scalar_tensor_tensor(
                    out=u[:], in0=tmp[:], scalar=1.5, in1=u[:],
                    op0=OP.add, op1=OP.mult,
                )

            zn = znpool.tile([P, TPS, D], BF16)
            nc.vector.tensor_mul(zn[:], zt[:], _broadcast_last(u[:], D))
            # batched xbar transpose for the whole 1024-col sub-chunk:
            # out[a, b, c] = in[c, b*128 + a]  ->  rt[d, t*128+p] = zn[p, t, d]
            nc.sync.dma_start(
                out=rt[:, c * SCR : (c + 1) * SCR].rearrange(
                    "d (t p) -> d t p", p=P
                ),
                in_=zn[:],
                transpose=True,
            )

        # ---- main loop: sim chunk -> exp -> row-sum ----
        expool = ctx.enter_context(tc.tile_pool(name="expool", bufs=4))
        for g in range(G):
            for rb in range(RB):
                ps = psum.tile([P, CH], F32)
                for s in range(CH // MM_N):
                    nc.tensor.matmul(
                        ps[:, s * MM_N : (s + 1) * MM_N],
                        rt[:, rb * P : (rb + 1) * P],
                        rt[:, g * CH + s * MM_N : g * CH + (s + 1) * MM_N],
                        start=True,
                        stop=True,
                    )
                if g in (0, 2):
                    # chunks carrying the self/pair diagonal: exp to SBUF
                    # scratch (PSUM frees immediately; extraction decouples)
                    ex = expool.tile([P, CH], F32)
                    nc.scalar.activation(
                        out=ex[:],
                        in_=ps[:],
                        func=AF.Exp,
                        scale=2.0,
                        accum_out=Ssum[:, rb * G + g : rb * G + g + 1],
                    )
                    # diag extract off the critical path: multiply the
                    # 128x128 diag block by identity, reduce along free
                    dst = (d8 if g == 0 else p8)[:, rb : rb + 1]
                    dummy = small.tile([P, P], F32)
                    nc.vector.tensor_mul(
                        dummy[:], ex[:, rb * P : rb * P + P], sb_ident[:]
                    )
                    nc.vector.tensor_reduce(dst, dummy[:], axis=AX.X, op=OP.add)
                else:
                    nc.scalar.activation(
                        out=ps[:],
                        in_=ps[:],
                        func=AF.Exp,
                        scale=2.0,
                        accum_out=Ssum[:, rb * G + g : rb * G + g + 1],
                    )

        # ---- epilogue ----
        S8 = singles.tile([P, RB], F32)
        nc.vector.tensor_reduce(
            S8[:], Ssum[:].rearrange("p (r g) -> p r g", g=G), axis=AX.X, op=OP.add
        )
        # S8 <- S8 - d8 + p8
        nc.vector.scalar_tensor_tensor(
            out=S8[:], in0=d8[:], scalar=-1.0, in1=S8[:], op0=OP.mult, op1=OP.add
        )
        nc.vector.tensor_add(S8[:], S8[:], p8[:])
        lse = singles.tile([P, RB], F32)
        nc.scalar.activation(out=lse[:], in_=S8[:], func=AF.Ln)
        p2 = singles.tile([P, RB], F32)
        nc.scalar.activation(out=p2[:], in_=p8[:], func=AF.Ln)
        loss8 = singles.tile([P, RB], F32)
        nc.vector.scalar_tensor_tensor(
            out=loss8[:], in0=p2[:], scalar=-1.0, in1=lse[:], op0=OP.mult, op1=OP.add
        )
        nc.sync.dma_start(out=out, in_=loss8[:])

    nc.compile()
    return nc


def get_nc():
    if "nc" not in _CACHE:
        _CACHE["nc"] = _build_nc()
    return _CACHE["nc"]


def make_in_maps(z_i: np.ndarray, z_j: np.ndarray):
    Z = np.concatenate(
        [
            np.asarray(z_i, np.float32).reshape(NROWS // 2, D),
            np.asarray(z_j, np.float32).reshape(NROWS // 2, D),
        ],
        axis=0,
    )
    ident = np.eye(P, dtype=np.float32)
    in_maps = []
    for k in range(N_CORES):
        zk = np.roll(Z, -k * ROWS_PER_CORE, axis=0)
        # z_sh[p, t*128+d] = zk[t*128+p, d]: contiguous per-partition DMA
        zsh = np.ascontiguousarray(
            zk.reshape(NROWS // P, P, D).transpose(1, 0, 2)
        ).reshape(P, NROWS)
        in_maps.append({"z": zsh, "ident": ident})
    return in_maps


def run_full(z_i: np.ndarray, z_j: np.ndarray, trace: bool = False):
    nc = get_nc()
    in_maps = make_in_maps(z_i, z_j)
    res = run_bass_kernel_spmd(nc, in_maps, list(range(N_CORES)), trace=trace)
    total = 0.0
    for k in range(N_CORES):
        total += float(np.asarray(res.results[k]["loss8"], np.float64).sum())
    loss = np.float32(total / NROWS)
    return loss, res


def kernel(z_i: np.ndarray, z_j: np.ndarray) -> np.ndarray:
    loss, _ = run_full(z_i, z_j, trace=False)
    return np.asarray(loss, dtype=np.float32)


# revision 26
# speedup vs baseline: 1.2034x; 1.0439x over previous
"""NT-Xent loss kernel for 8 Trainium2 NeuronCores.

Math (matches the reference):
  Z = concat(z_i, z_j).reshape(8192, 128); r = row-l2-normalize(Z)
  sim = r @ r.T                                  (8192 x 8192)
  row i: S_i   = sum_j exp(2*sim[i, j])          (full row, incl. self)
         d_i   = exp(2*sim[i, i])                (self term)
         p_i   = exp(2*sim[i, pair(i)]),  pair(i) = (i + 4096) % 8192
  loss_i = log(S_i - d_i + p_i) - log(p_i)
  loss   = mean_i(loss_i)

Sharding: rows are split across 8 cores (1024 rows each). Every core gets
the full Z, but ROTATED so its own rows come first; this makes the
self-diagonal land at local columns [0, 1024) and the pair diagonal at
[4096, 5120) on every core, so one SPMD program works for all cores.
Each core emits its 1024 per-row losses; the host sums them (the scalar
all-reduce) and divides by 2N.

Host-side staging: z is supplied pre-rotated AND pre-tiled as
z_sh[p, t*128 + d] = z_rot[t*128 + p, d] so every DMA is fully
contiguous per partition.

Per-core pipeline:
  1. DMA z (fp32) in 8 sub-chunks of 1024 rows.
  2. Row norms on DVE only: square, reduce, rsqrt via Quake seed + 2
     fused Newton steps (no ACT Sqrt -> no activation-table thrash).
  3. Scale rows to unit norm, cast bf16, one batched DMA-xbar transpose
     per sub-chunk into RT[d, 8192].
  4. Main loop: per (2048-col chunk x 128-row block): 4 bf16 matmuls
     (512 cols each) into PSUM, one ACT Exp (scale=2) with fused row-sum
     (accum_out); diag-carrying chunks write exp to SBUF scratch so the
     PSUM slot frees immediately and the diag extraction (multiply by
     identity + reduce on DVE) runs off the critical path.
  5. Epilogue: S - d + p, Ln, subtract, DMA out [128, 8] losses.
"""

import sys

import numpy as np

sys.path.insert(0, "/opt/trn_rl_repo")

from contextlib import ExitStack  # noqa: E402

import concourse.bass as bass  # noqa: E402
import concourse.tile as tile  # noqa: E402
from concourse import bacc, mybir  # noqa: E402
from concourse.bass_utils import run_bass_kernel_spmd  # noqa: E402

P = 128
N_CORES = 8
NROWS = 8192  # 2N
D = 128
ROWS_PER_CORE = NROWS // N_CORES  # 1024
RB = ROWS_PER_CORE // P  # 8 row blocks per core
G = 4  # column chunk groups (main loop)
CH = NROWS // G  # 2048 columns per chunk
SC = 8  # normalization sub-chunks
SCR = NROWS // SC  # 1024 rows per sub-chunk
TPS = SCR // P  # 8 row-tiles per sub-chunk
MM_N = 512  # matmul moving free dim (one PSUM bank)

F32 = mybir.dt.float32
BF16 = mybir.dt.bfloat16
U32 = mybir.dt.uint32
AF = mybir.ActivationFunctionType
OP = mybir.AluOpType
AX = mybir.AxisListType

_CACHE = {}


def _broadcast_last(ap: bass.AP, n: int) -> bass.AP:
    """Append a stride-0 dim of size n to an AP (free-axis broadcast)."""
    return bass.AP(tensor=ap.tensor, offset=ap.offset, ap=[*ap.ap, [0, n]])


def _build_nc():
    nc = bacc.Bacc(
        "TRN2", target_bir_lowering=False, debug=False, num_devices=N_CORES
    )
    z = nc.dram_tensor("z", [P, NROWS], F32, kind="ExternalInput").ap()
    ident = nc.dram_tensor("ident", [P, P], F32, kind="ExternalInput").ap()
    out = nc.dram_tensor("loss8", [P, RB], F32, kind="ExternalOutput").ap()

    with tile.TileContext(nc) as tc, ExitStack() as ctx:
        zpool = ctx.enter_context(tc.tile_pool(name="zpool", bufs=SC))
        sqpool = ctx.enter_context(tc.tile_pool(name="sqpool", bufs=3))
        znpool = ctx.enter_context(tc.tile_pool(name="znpool", bufs=4))
        small = ctx.enter_context(tc.tile_pool(name="small", bufs=4))
        singles = ctx.enter_context(tc.tile_pool(name="singles", bufs=1))
        psum = ctx.enter_context(tc.tile_pool(name="psum", bufs=2, space="PSUM"))

        # Persistent transposed normalized representation: RT[d, n]
        rt = singles.tile([P, NROWS], BF16)

        Ssum = singles.tile([P, RB * G], F32)  # per (row, chunk) partial sums
        d8 = singles.tile([P, RB], F32)  # exp(2*self)
        p8 = singles.tile([P, RB], F32)  # exp(2*pair)

        # ---- normalization: 8 pipelined sub-chunks of 1024 rows ----
        # All loads are emitted first so no queue-blocking wait (e.g. an
        # xbar transpose waiting on zn) can delay a later load's dispatch.
        zts = []
        for c in range(SC):
            zt = zpool.tile([P, TPS, D], F32)
            if c == 0:
                # split the first (critical-path) load into two parallel DMAs
                half = SCR // 2
                nc.sync.dma_start(out=zt[:, : TPS // 2, :], in_=z[:, 0:half])
                nc.sync.dma_start(out=zt[:, TPS // 2 :, :], in_=z[:, half:SCR])
            else:
                nc.sync.dma_start(out=zt[:], in_=z[:, c * SCR : (c + 1) * SCR])
            zts.append(zt)
        sb_ident = singles.tile([P, P], F32)
        nc.sync.dma_start(out=sb_ident[:], in_=ident)

        for c in range(SC):
            zt = zts[c]
            # fused square + row-sum: per tile one scalar_tensor_tensor with
            # accum_out (out = z*z is scratch, accum = sum over free axis)
            sq = sqpool.tile([P, TPS, D], F32)
            ss = small.tile([P, TPS], F32)
            for t in range(TPS):
                nc.vector.scalar_tensor_tensor(
                    out=sq[:, t, :],
                    in0=zt[:, t, :],
                    scalar=1.0,
                    in1=zt[:, t, :],
                    op0=OP.mult,
                    op1=OP.mult,
                    accum_out=ss[:, t : t + 1],
                )

            # u = 1/sqrt(ss): Quake seed + 2 fused Newton iterations.
            # DVE's scalar ALU promotes to f32, so build the seed as
            # (0xBE6EB3BE - bits) via float mult/add, then integer >>1.
            u = small.tile([P, TPS], F32)
            tmp = small.tile([P, TPS], F32)
            nc.vector.tensor_scalar(
                tmp[:].bitcast(U32),
                ss[:].bitcast(U32),
                -1.0,
                float(0xBE6EB3BE),
                OP.mult,
                OP.add,
            )
            nc.vector.tensor_scalar(
                u[:].bitcast(U32),
                tmp[:].bitcast(U32),
                1,
                None,
                OP.logical_shift_right,
            )
            for _ in range(2):
                # t = (y*y * -0.5) * ss ; y = (t + 1.5) * y
                nc.vector.tensor_mul(tmp[:], u[:], u[:])
                nc.vector.scalar_tensor_tensor(
                    out=tmp[:], in0=tmp[:], scalar=-0.5, in1=ss[:],
                    op0=OP.mult, op1=OP.mult,
                )
                nc.vector.scalar_tensor_tensor(
                    out=u[:], in0=tmp[:], scalar=1.5, in1=u[:],
                    op0=OP.add, op1=OP.mult,
                )

            zn = znpool.tile([P, TPS, D], BF16)
            nc.vector.tensor_mul(zn[:], zt[:], _broadcast_last(u[:], D))
            # batched xbar transpose for the whole 1024-col sub-chunk:
            # out[a, b, c] = in[c, b*128 + a]  ->  rt[d, t*128+p] = zn[p, t, d]
            nc.sync.dma_start(
                out=rt[:, c * SCR : (c + 1) * SCR].rearrange(
                    "d (t p) -> d t p", p=P
                ),
                in_=zn[:],
                transpose=True,
            )

        # ---- main loop: sim chunk -> exp -> row-sum ----
        expool = ctx.enter_context(tc.tile_pool(name="expool", bufs=6))
        for g in range(G):
            for rb in range(RB):
                ps = psum.tile([P, CH], F32)
                for s in range(CH // MM_N):
                    nc.tensor.matmul(
                        ps[:, s * MM_N : (s + 1) * MM_N],
                        rt[:, rb * P : (rb + 1) * P],
                        rt[:, g * CH + s * MM_N : g * CH + (s + 1) * MM_N],
                        start=True,
                        stop=True,
                    )
                if g in (0, 2):
                    # chunks carrying the self/pair diagonal: exp to SBUF
                    # scratch (PSUM frees immediately; extraction decouples)
                    ex = expool.tile([P, CH], F32)
                    nc.scalar.activation(
                        out=ex[:],
                        in_=ps[:],
                        func=AF.Exp,
                        scale=2.0,
                        accum_out=Ssum[:, rb * G + g : rb * G + g + 1],
                    )
                    # diag extract off the critical path: multiply the
                    # 128x128 diag block by identity, reduce along free
                    dst = (d8 if g == 0 else p8)[:, rb : rb + 1]
                    dummy = small.tile([P, P], F32)
                    nc.vector.tensor_mul(
                        dummy[:], ex[:, rb * P : rb * P + P], sb_ident[:]
                    )
                    nc.vector.tensor_reduce(dst, dummy[:], axis=AX.X, op=OP.add)
                else:
                    nc.scalar.activation(
                        out=ps[:],
                        in_=ps[:],
                        func=AF.Exp,
                        scale=2.0,
                        accum_out=Ssum[:, rb * G + g : rb * G + g + 1],
                    )

        # ---- epilogue ----
        S8 = singles.tile([P, RB], F32)
        nc.vector.tensor_reduce(
            S8[:], Ssum[:].rearrange("p (r g) -> p r g", g=G), axis=AX.X, op=OP.add
        )
        # S8 <- S8 - d8 + p8
        nc.vector.scalar_tensor_tensor(
            out=S8[:], in0=d8[:], scalar=-1.0, in1=S8[:], op0=OP.mult, op1=OP.add
        )
        nc.vector.tensor_add(S8[:], S8[:], p8[:])
        lse = singles.tile([P, RB], F32)
        nc.scalar.activation(out=lse[:], in_=S8[:], func=AF.Ln)
        p2 = singles.tile([P, RB], F32)
        nc.scalar.activation(out=p2[:], in_=p8[:], func=AF.Ln)
        loss8 = singles.tile([P, RB], F32)
        nc.vector.scalar_tensor_tensor(
            out=loss8[:], in0=p2[:], scalar=-1.0, in1=lse[:], op0=OP.mult, op1=OP.add
        )
        nc.sync.dma_start(out=out, in_=loss8[:])

    nc.compile()
    return nc


def get_nc():
    if "nc" not in _CACHE:
        _CACHE["nc"] = _build_nc()
    return _CACHE["nc"]


def make_in_maps(z_i: np.ndarray, z_j: np.ndarray):
    Z = np.concatenate(
        [
            np.asarray(z_i, np.float32).reshape(NROWS // 2, D),
            np.asarray(z_j, np.float32).reshape(NROWS // 2, D),
        ],
        axis=0,
    )
    ident = np.eye(P, dtype=np.float32)
    in_maps = []
    for k in range(N_CORES):
        zk = np.roll(Z, -k * ROWS_PER_CORE, axis=0)
        # z_sh[p, t*128+d] = zk[t*128+p, d]: contiguous per-partition DMA
        zsh = np.ascontiguousarray(
            zk.reshape(NROWS // P, P, D).transpose(1, 0, 2)
        ).reshape(P, NROWS)
        in_maps.append({"z": zsh, "ident": ident})
    return in_maps


def run_full(z_i: np.ndarray, z_j: np.ndarray, trace: bool = False):
    nc = get_nc()
    in_maps = make_in_maps(z_i, z_j)
    res = run_bass_kernel_spmd(nc, in_maps, list(range(N_CORES)), trace=trace)
    total = 0.0
    for k in range(N_CORES):
        total += float(np.asarray(res.results[k]["loss8"], np.float64).sum())
    loss = np.float32(total / NROWS)
    return loss, res


def kernel(z_i: np.ndarray, z_j: np.ndarray) -> np.ndarray:
    loss, _ = run_full(z_i, z_j, trace=False)
    return np.asarray(loss, dtype=np.float32)
